# revision 22
# baseline (speedup 1.0000x reference)
"""Exponentiated-quadratic (RBF) kernel matrix on 8 Trainium2 NeuronCores.

K[i, j] = sigma * exp(-0.5 * ||x1_i/rho - x2_j/rho||^2)
        with sigma = exp(log_sigma)^2, rho = exp(log_rho)

Strategy
--------
Row-shard x1 across the 8 cores (512 rows each), replicate x2. The squared
distance is folded into a single augmented matmul: with
  a_i = [x1_i/rho, -0.5*||x1_i/rho||^2, 1]          (P+2 = 34 wide)
  b_j = [x2_j/rho, 1, -0.5*||x2_j/rho||^2]
the dot product a_i . b_j = -0.5 * d_ij, so the whole epilogue is one
ScalarE activation: K = exp(s + 2*log_sigma)  (ACT computes func(x*scale+bias)
for free). Host prep is only the tiny (N,P) scaling/transpose; all O(N*M)
work (matmul, exp, output traffic) runs on-device.

Per core: 4 row-blocks of 128 x 4096. Each row-block is 8 matmuls
(K=34, 128x512 out) into two 4-bank PSUM tiles, 2 exp-activations
PSUM->SBUF, then one 2 MiB DMA to HBM. The kernel is output-DMA bound
(8 MiB/core at ~360 GB/s).
"""

import os

import numpy as np

import concourse.bass as bass
import concourse.mybir as mybir
import concourse.tile as tile
from concourse.bass_utils import run_bass_kernel_spmd
from concourse.tile import add_dep_helper

N, M, P = 4096, 4096, 32
NCORES = 8
NSHARD = N // NCORES  # 512 rows of x1 per core
KAUG = P + 2          # 34: contraction dim after augmentation
IBLK = 128            # output row-block = PSUM partition dim
JBLK = 512            # matmul free dim = one fp32 PSUM bank
PSW = 2048            # PSUM tile width (4 banks) = one exp-activation
BCH = 1024            # x2 load chunk width (pipelines load under compute)

# matmul input interpretation: "fp32r" (1 cycle/row, relaxed precision),
# "fp32" (4 cycles/row, full precision)
MM_MODE = os.environ.get("BASS_MM_MODE", "fp32r")


def _build_nc(bias_val: float):
    nc = bass.Bass()
    mm_dt = mybir.dt.float32r if MM_MODE == "fp32r" else mybir.dt.float32
    # ab_t = [A_shard | B[:, :BCH]] fused so the first matmul depends on ONE
    # DMA (walrus rejects matmuls carrying multiple sync waits), then the
    # remaining B chunks pipeline in with one new wait each.
    ab_t = nc.declare_dram_parameter(
        "ab_t", [KAUG, NSHARD + BCH], mm_dt, isOutput=False
    )
    b_t = nc.declare_dram_parameter("b_t", [KAUG, M - BCH], mm_dt, isOutput=False)
    out = nc.declare_dram_parameter("out", [NSHARD, M], mybir.dt.float32, isOutput=True)

    with tile.TileContext(nc) as tc:
        with (
            tc.tile_pool(name="inp", bufs=1) as inp_pool,
            tc.tile_pool(name="stage", bufs=1) as stage_pool,
            tc.tile_pool(name="ps", bufs=1, space="PSUM") as ps_pool,
        ):
            dma_insts = []
            ab_sb = inp_pool.tile([KAUG, NSHARD + BCH], mm_dt, tag="ab")
            dma_insts.append(nc.sync.dma_start(out=ab_sb, in_=ab_t[:, :]))
            # b chunk c covers output cols [c*BCH, (c+1)*BCH); chunk 0 lives
            # in ab_sb at offset NSHARD.
            b_chunks = [ab_sb[:, NSHARD : NSHARD + BCH]]
            for c in range(1, M // BCH):
                b_sb = inp_pool.tile([KAUG, BCH], mm_dt, tag=f"b{c}")
                dma_insts.append(
                    nc.sync.dma_start(
                        out=b_sb, in_=b_t[:, (c - 1) * BCH : c * BCH]
                    )
                )
                b_chunks.append(b_sb[:, :])

            # Persistent PSUM tiles (allocated once, reused across row-blocks)
            # so the slot-reuse dep is a single in-tile WAR wait on ACT; pool
            # re-allocation would emit an extra same-engine PE wait that the
            # fused fp32 matmul instruction cannot carry (walrus 1-wait limit).
            ps_tiles = [
                ps_pool.tile(
                    [IBLK, PSW], mybir.dt.float32, tag=f"ps{h}", name=f"ps{h}"
                )
                for h in range(M // PSW)
            ]
            act_insts = []
            mm_insts = []
            for i in range(NSHARD // IBLK):  # 4 row-blocks
                # one staging buffer per row-block (no reuse): keeps the ACT
                # instruction at a single sync wait (PE) — walrus rejects
                # multi-wait ACTIVATE/Matmult instructions.
                out_sb = stage_pool.tile(
                    [IBLK, M], mybir.dt.float32, tag=f"out{i}", name=f"out{i}"
                )
                lhsT = ab_sb[:, i * IBLK : (i + 1) * IBLK]
                for h in range(M // PSW):  # 2 PSUM tiles per row-block
                    ps = ps_tiles[h]
                    for q in range(PSW // JBLK):  # 4 matmuls per PSUM tile
                        j = h * (PSW // JBLK) + q
                        bch = b_chunks[j * JBLK // BCH]
                        boff = (j * JBLK) % BCH
                        mm_insts.append(
                            nc.tensor.matmul(
                                ps[:, q * JBLK : (q + 1) * JBLK],
                                lhsT=lhsT,
                                rhs=bch[:, boff : boff + JBLK],
                                start=True,
                                stop=True,
                            )
                        )
                    act_insts.append(
                        nc.scalar.activation(
                            out=out_sb[:, h * PSW : (h + 1) * PSW],
                            in_=ps,
                            func=mybir.ActivationFunctionType.Exp,
                            bias=float(bias_val),
                            scale=1.0,
                        )
                    )
                dma_insts.append(
                    nc.sync.dma_start(
                        out=out[i * IBLK : (i + 1) * IBLK, :], in_=out_sb
                    )
                )

            # ACT->ACT sync deps come from PSUM bank-pair serialization of two
            # READS of the same tile — already transitively ordered through
            # the interleaved matmuls, and same-engine FIFO besides. Demote to
            # nosync (ordering-only): walrus rejects ACTIVATE instructions
            # carrying more than one semaphore wait.
            import bass_rust as _br

            act_names = {a.ins.name for a in act_insts}
            for a in act_insts:
                deps = list(a.ins.sync_dependency_names())
                spurious = [d for d in deps if d in act_names]
                if spurious:
                    keep = [d for d in deps if d not in act_names]
                    a.ins.take_sync_dependencies()
                    a.ins.set_sync_dependencies(
                        _br.InstructionNameOrderedSet(keep)
                    )
                    a.ins.add_nosync_dependencies_from(
                        _br.InstructionNameOrderedSet(spurious)
                    )

            # Wait-funnel for the kernel-tail drain: the framework drain waits
            # on every live semaphore at once, which walrus rejects (sync-wait
            # slot limit). Observe each completion on the SP sequencer via
            # single-wait nops first, so the drain itself needs no waits.
            for t in [mm_insts[-1], act_insts[-1], *dma_insts]:
                nop = nc.sync.nop(nofuse=True, hint="tail_funnel")
                add_dep_helper(nop.ins, t.ins, True, "tail wait funnel")
    return nc


def run(x1, x2, log_rho, log_sigma, trace=False):
    """Returns (K, exec_time_ns). exec_time_ns is None unless trace=True."""
    x1 = np.asarray(x1, dtype=np.float32)
    x2 = np.asarray(x2, dtype=np.float32)
    rho = float(np.exp(np.float64(np.asarray(log_rho))))
    bias = 2.0 * float(np.asarray(log_sigma))  # log(sigma) = 2*log_sigma

    xs = (x1 / np.float32(rho)).astype(np.float32)
    ys = (x2 / np.float32(rho)).astype(np.float32)
    xn = np.einsum("np,np->n", xs, xs, dtype=np.float64)
    yn = np.einsum("mp,mp->m", ys, ys, dtype=np.float64)

    a_full = np.empty((KAUG, N), np.float32)
    a_full[:P] = xs.T
    a_full[P] = (-0.5 * xn).astype(np.float32)
    a_full[P + 1] = 1.0
    b_full = np.empty((KAUG, M), np.float32)
    b_full[:P] = ys.T
    b_full[P] = 1.0
    b_full[P + 1] = (-0.5 * yn).astype(np.float32)

    nc = _build_nc(bias)
    b_rest = np.ascontiguousarray(b_full[:, BCH:])
    in_maps = [
        {
            "ab_t": np.ascontiguousarray(
                np.concatenate(
                    [a_full[:, c * NSHARD : (c + 1) * NSHARD], b_full[:, :BCH]],
                    axis=1,
                )
            ),
            "b_t": b_rest,
        }
        for c in range(NCORES)
    ]
    res = run_bass_kernel_spmd(
        nc, in_maps, core_ids=list(range(NCORES)), trace=trace
    )
    full = np.concatenate(
        [res.results[c]["out"] for c in range(NCORES)], axis=0
    )
    return full, res.exec_time_ns


def kernel(x1, x2, log_rho, log_sigma):
    out, _ = run(x1, x2, log_rho, log_sigma, trace=False)
    return out
